# revision 43
# baseline (speedup 1.0000x reference)
"""GCNConv (PyG-style) distributed Bass kernel for 8 TRN2 NeuronCores.

Strategy (edge-parallel by destination, node-partitioned output):
  - Host: deg/dinv via bincount; x' owner-computed on device:
      x'[v] = dinv[v] * (feature[v] @ W)   (per-core node shard, 98 blocks)
  - AllGather x' across the 8 cores (collective_compute).
  - Edges bucketed on host by destination block (128 dest nodes per block).
    Per block: one indirect-DMA gather of the source rows of x', a one-hot
    selection matrix (is_equal vs iota), and matmul-accumulation into PSUM
    implements an exact scatter-add. Self-loop added via identity matmul.
  - Epilogue: *dinv, +b, ReLU -> x_out rows DMA'd out.
  - h = sigmoid(mean(x_out)) computed on host from the full output.
"""

import sys
from contextlib import ExitStack

import numpy as np

if "/opt/trn_rl_repo" not in sys.path:
    sys.path.insert(0, "/opt/trn_rl_repo")

P = 128

FULL_CFG = dict(
    N=100000,
    FIN=256,
    FOUT=128,
    NCORES=8,
)

TRACE = False
LAST_EXEC_NS = None
LAST_RESULTS = None
BF16_TABLE = True   # x_loc/x_gath/gather in bf16
BF16_MM = True      # phase-3 matmul operands (oh/gt/ident/xblk) in bf16


def _derive(cfg):
    c = dict(cfg)
    c["NPC"] = -(-c["N"] // c["NCORES"])          # valid nodes per core (ceil)
    c["NBLK"] = -(-c["NPC"] // P)                  # 128-node blocks per core
    c["NPC_PAD"] = c["NBLK"] * P
    c["N_PAD"] = c["NCORES"] * c["NPC_PAD"]
    return c


def preprocess(feature, edge_index, W, b, cfg):
    cfg = _derive(cfg)
    N, NCORES, NPC, NBLK, NPC_PAD = (
        cfg["N"], cfg["NCORES"], cfg["NPC"], cfg["NBLK"], cfg["NPC_PAD"],
    )
    FIN, FOUT = cfg["FIN"], cfg["FOUT"]

    feature = np.ascontiguousarray(feature, dtype=np.float32)
    W = np.ascontiguousarray(W, dtype=np.float32)
    b = np.ascontiguousarray(b, dtype=np.float32)
    row = edge_index[0].astype(np.int64)
    col = edge_index[1].astype(np.int64)

    deg = np.bincount(col, minlength=N).astype(np.float32) + 1.0
    dinv = (1.0 / np.sqrt(deg)).astype(np.float32)

    blk_g = (col // NPC) * NBLK + (col % NPC) // P     # global dest block id
    slot = (col % NPC) % P
    g_of = (row // NPC) * NPC_PAD + (row % NPC)        # gather row (padded layout)

    order = np.argsort(blk_g, kind="stable")
    slot_s = slot[order].astype(np.float32)
    g_s = g_of[order].astype(np.int32)

    cnt = np.bincount(blk_g, minlength=NCORES * NBLK).reshape(NCORES, NBLK)
    Ts = [int(np.ceil(cnt[:, i] / P).max()) for i in range(NBLK)]
    offs = np.concatenate([[0], np.cumsum(Ts)]).astype(int)
    TT = int(offs[-1])

    gidx_h = np.zeros((NCORES, P, max(TT, 1)), np.int32)
    dloc_h = np.full((NCORES, P, max(TT, 1)), -1.0, np.float32)
    bounds = np.concatenate([[0], np.cumsum(cnt.ravel())]).astype(np.int64)
    for c in range(NCORES):
        for i in range(NBLK):
            b0, b1 = bounds[c * NBLK + i], bounds[c * NBLK + i + 1]
            k = b1 - b0
            if k == 0:
                continue
            js = np.arange(k)
            gidx_h[c, js % P, offs[i] + js // P] = g_s[b0:b1]
            dloc_h[c, js % P, offs[i] + js // P] = slot_s[b0:b1]

    bb = np.tile(b[None, :], (P, 1)).astype(np.float32)

    in_maps = []
    for c in range(NCORES):
        lo = c * NPC
        hi = min(lo + NPC, N)
        v = max(hi - lo, 0)
        ft = np.zeros((FIN, NPC_PAD), np.float32)
        if v > 0:
            ft[:, :v] = feature[lo:hi].T
        # pack so each block's [P, 2P] lhsT tile is one contiguous DMA read:
        # ftall[p, i*2P + j*P + c] = featT[j*P + p, i*P + c]
        ftall = np.ascontiguousarray(
            ft.reshape(FIN // P, P, NBLK, P)
            .transpose(1, 2, 0, 3)
            .reshape(P, NBLK * (FIN // P) * P)
        )
        dl = np.zeros(NPC_PAD, np.float32)
        if v > 0:
            dl[:v] = dinv[lo:hi]
        in_maps.append({
            "ftall": ftall,
            "w": W,
            "dinv": np.ascontiguousarray(dl.reshape(NBLK, P).T),
            "bb": bb,
            "gidx": np.ascontiguousarray(gidx_h[c]),
            "dloc": np.ascontiguousarray(dloc_h[c]),
        })
    return in_maps, Ts, cfg


def build_program(cfg, Ts, debug=False):
    import concourse.bacc as bacc
    import concourse.bass as bass
    import concourse.mybir as mybir
    import concourse.tile as tile
    from concourse.masks import make_identity

    cfg = _derive(cfg)
    NCORES, NBLK, NPC_PAD, N_PAD = (
        cfg["NCORES"], cfg["NBLK"], cfg["NPC_PAD"], cfg["N_PAD"],
    )
    FIN, FOUT = cfg["FIN"], cfg["FOUT"]

    Ts = list(Ts)
    offs = np.concatenate([[0], np.cumsum(Ts)]).astype(int)
    TT = max(int(offs[-1]), 1)
    TMAX = max(max(Ts), 1)

    f32 = mybir.dt.float32
    i32 = mybir.dt.int32
    bf16 = mybir.dt.bfloat16
    tdt = bf16 if BF16_TABLE else f32
    mdt = bf16 if BF16_MM else f32

    nc = bacc.Bacc(num_devices=NCORES, num_swdge_queues=4)
    ftall = nc.declare_dram_parameter(
        "ftall", [P, NBLK * (FIN // P) * P], f32, isOutput=False
    )
    w = nc.declare_dram_parameter("w", [FIN, FOUT], f32, isOutput=False)
    dinv = nc.declare_dram_parameter("dinv", [P, NBLK], f32, isOutput=False)
    bb = nc.declare_dram_parameter("bb", [P, FOUT], f32, isOutput=False)
    gidx = nc.declare_dram_parameter("gidx", [P, TT], i32, isOutput=False)
    dloc = nc.declare_dram_parameter("dloc", [P, TT], f32, isOutput=False)
    xout = nc.declare_dram_parameter("xout", [NPC_PAD, FOUT], f32, isOutput=True)
    if debug:
        dbg_iota = nc.declare_dram_parameter(
            "dbg_iota", [P, TMAX, P], f32, isOutput=True)
        dbg_oh = nc.declare_dram_parameter(
            "dbg_oh", [P, TMAX, P], mdt, isOutput=True)
        dbg_gt = nc.declare_dram_parameter(
            "dbg_gt", [P, TMAX, P], mdt, isOutput=True)
        dbg_xall = nc.declare_dram_parameter(
            "dbg_xall", [P, NBLK * FOUT], f32, isOutput=True)
        dbg_xg = nc.declare_dram_parameter(
            "dbg_xg", [4 * P, FOUT], f32, isOutput=True)

    x_loc = nc.dram_tensor("x_loc", [NPC_PAD, FOUT], tdt, kind="Internal")
    x_gath = nc.dram_tensor(
        "x_gath", [N_PAD, FOUT], tdt, kind="Internal", addr_space="Shared"
    )

    with ExitStack() as ctx:
        tc = ctx.enter_context(tile.TileContext(nc))
        const_tp = ctx.enter_context(tc.tile_pool(name="const", bufs=1))
        work_tp = ctx.enter_context(tc.tile_pool(name="work", bufs=2))
        big_tp = ctx.enter_context(tc.tile_pool(name="big", bufs=2))
        psum_tp = ctx.enter_context(tc.tile_pool(name="psum", bufs=2, space="PSUM"))

        # Raw DMA-landed constants.  Every operand a compute engine reads is
        # then re-materialized by a DVE copy: engine instruction structs fit
        # only ONE sync-wait command, so all compute-side deps must coalesce
        # onto a single semaphore (DVE's), with DMA deps carried by the
        # copies (DMACopy descriptors allow multiple waits).
        w_raw = const_tp.tile([P, 2 * FOUT], f32)
        nc.sync.dma_start(out=w_raw[:, 0:FOUT], in_=w[0:P, :])
        nc.sync.dma_start(out=w_raw[:, FOUT:2 * FOUT], in_=w[P:FIN, :])
        dinv_raw = const_tp.tile([P, NBLK], f32)
        nc.sync.dma_start(out=dinv_raw[:], in_=dinv[:, :])
        bb_raw = const_tp.tile([P, FOUT], f32)
        nc.sync.dma_start(out=bb_raw[:], in_=bb[:, :])
        gidx_sb = const_tp.tile([P, TT], i32)
        nc.sync.dma_start(out=gidx_sb[:], in_=gidx[:, :])
        dloc_raw = const_tp.tile([P, TT], f32)
        nc.sync.dma_start(out=dloc_raw[:], in_=dloc[:, :])
        ident_raw = const_tp.tile([P, P], f32)
        make_identity(nc, ident_raw[:])

        w_sb = const_tp.tile([P, 2 * FOUT], f32)
        nc.vector.tensor_copy(out=w_sb[:], in_=w_raw[:])
        dinv_sb = const_tp.tile([P, NBLK], f32)
        nc.vector.tensor_copy(out=dinv_sb[:], in_=dinv_raw[:])
        bb_sb = const_tp.tile([P, FOUT], f32)
        nc.vector.tensor_copy(out=bb_sb[:], in_=bb_raw[:])
        dloc_sb = const_tp.tile([P, TT], mdt)
        nc.vector.tensor_copy(out=dloc_sb[:], in_=dloc_raw[:])
        ident = const_tp.tile([P, P], mdt)
        nc.vector.tensor_copy(out=ident[:], in_=ident_raw[:])
        iota_i = const_tp.tile([P, TMAX, P], i32)
        nc.gpsimd.iota(iota_i[:], pattern=[[0, TMAX], [1, P]], base=0,
                       channel_multiplier=0)
        iota_f = const_tp.tile([P, TMAX, P], f32)
        nc.vector.tensor_copy(out=iota_f[:], in_=iota_i[:])
        iota_m = const_tp.tile([P, TMAX, P], mdt)
        nc.vector.tensor_copy(out=iota_m[:], in_=iota_f[:])
        xall = const_tp.tile([P, NBLK * FOUT], f32)

        KT = FIN // P

        # phase 1: x'-blocks = dinv * (feat @ W)
        for i in range(NBLK):
            ftr = work_tp.tile([P, KT * P], f32, tag="ftr")
            nc.sync.dma_start(out=ftr[:], in_=ftall[:, i * KT * P:(i + 1) * KT * P])
            ft = work_tp.tile([P, KT * P], f32, tag="ft")
            nc.vector.tensor_copy(out=ft[:], in_=ftr[:])
            px = psum_tp.tile([P, FOUT], f32, tag="px")
            nc.tensor.matmul(out=px[:], lhsT=ft[:, 0:P], rhs=w_sb[:, 0:FOUT],
                             start=True, stop=False)
            nc.tensor.matmul(out=px[:], lhsT=ft[:, P:2 * P], rhs=w_sb[:, FOUT:2 * FOUT],
                             start=False, stop=True)
            xp = xall[:, i * FOUT:(i + 1) * FOUT]
            nc.vector.tensor_scalar_mul(out=xp, in0=px[:], scalar1=dinv_sb[:, i:i + 1])
            xbf = work_tp.tile([P, FOUT], tdt, tag="xbf")
            nc.vector.tensor_copy(out=xbf[:], in_=xp)
            nc.sync.dma_start(out=x_loc[i * P:(i + 1) * P, :], in_=xbf[:])

        # phase 2: allgather x' across cores
        nc.gpsimd.collective_compute(
            "AllGather",
            mybir.AluOpType.bypass,
            replica_groups=[list(range(NCORES))],
            ins=[x_loc[:, :]],
            outs=[x_gath[:, :]],
        )

        # phase 3: per dest block, gather + one-hot matmul scatter-add.
        # DVE order per block: is_equal -> xblk copy.  The identity matmul
        # reads xblk, so its single DVE wait also covers oh (is_equal), and
        # the t=0 scatter matmul then needs only the gather's DMASW wait.
        for i in range(NBLK):
            T = int(Ts[i])
            off = int(offs[i])
            ps = psum_tp.tile([P, FOUT], f32, tag="agg")
            if T > 0:
                oh = big_tp.tile([P, TMAX, P], mdt, tag="oh")
                nc.vector.tensor_tensor(
                    out=oh[:, 0:T, :],
                    in0=dloc_sb[:, off:off + T].to_broadcast([P, T, P]),
                    in1=iota_m[:, 0:T, :],
                    op=mybir.AluOpType.is_equal,
                )
            xblk = work_tp.tile([P, FOUT], mdt, tag="xblk")
            nc.vector.tensor_copy(out=xblk[:], in_=xall[:, i * FOUT:(i + 1) * FOUT])
            if T > 0:
                gt = big_tp.tile([P, TMAX, P], tdt, tag="gath")
                for t in range(T):
                    nc.gpsimd.indirect_dma_start(
                        out=gt[:, t, :],
                        out_offset=None,
                        in_=x_gath[:, :],
                        in_offset=bass.IndirectOffsetOnAxis(
                            ap=gidx_sb[:, off + t:off + t + 1], axis=0
                        ),
                    )
                if tdt != mdt:
                    gtm = big_tp.tile([P, TMAX, P], mdt, tag="gathm")
                    nc.vector.tensor_copy(out=gtm[:, 0:T, :], in_=gt[:, 0:T, :])
                    gt = gtm
            nc.tensor.matmul(out=ps[:], lhsT=ident[:], rhs=xblk[:],
                             start=True, stop=(T == 0))
            for t in range(T):
                nc.tensor.matmul(
                    out=ps[:],
                    lhsT=oh[:, t:t + 1, :],
                    rhs=gt[:, t:t + 1, :],
                    start=False,
                    stop=(t == T - 1),
                )
            if debug and i == 0:
                nc.sync.dma_start(out=dbg_iota[:, :, :], in_=iota_f[:])
                if T > 0:
                    nc.sync.dma_start(out=dbg_oh[:, 0:T, :], in_=oh[:, 0:T, :])
                    nc.sync.dma_start(out=dbg_gt[:, 0:T, :], in_=gt[:, 0:T, :])
                nc.sync.dma_start(out=dbg_xall[:, :], in_=xall[:])
                nc.sync.dma_start(out=dbg_xg[:, :], in_=x_gath[0:4 * P, :])
            t1 = work_tp.tile([P, FOUT], f32, tag="epi")
            nc.vector.tensor_scalar_mul(out=t1[:], in0=ps[:],
                                        scalar1=dinv_sb[:, i:i + 1])
            nc.vector.tensor_add(out=t1[:], in0=t1[:], in1=bb_sb[:])
            xo = work_tp.tile([P, FOUT], f32, tag="xo")
            nc.vector.tensor_scalar_max(out=xo[:], in0=t1[:], scalar1=0.0)
            nc.sync.dma_start(out=xout[i * P:(i + 1) * P, :], in_=xo[:])

    return nc


def postprocess(results, cfg):
    cfg = _derive(cfg)
    N, NCORES, NPC = cfg["N"], cfg["NCORES"], cfg["NPC"]
    parts = []
    for c in range(NCORES):
        lo = c * NPC
        hi = min(lo + NPC, N)
        if hi > lo:
            parts.append(np.asarray(results[c]["xout"])[: hi - lo])
    x_out = np.concatenate(parts, axis=0).astype(np.float32)
    mean = x_out.mean(axis=0, dtype=np.float64).astype(np.float32)
    h = (1.0 / (1.0 + np.exp(-mean.astype(np.float64)))).astype(np.float32)
    return x_out, h


def _setup_trace():
    """Register the NTFF profile hook that this image's antenv lacks."""
    import types

    try:
        from antenv.axon_hooks import get_axon_ntff_profile_hook  # noqa: F401
    except ImportError:
        import antenv

        mod = types.ModuleType("antenv.axon_hooks")
        _h = {"hook": None}
        mod.set_axon_ntff_profile_hook = lambda h: _h.__setitem__("hook", h)
        mod.get_axon_ntff_profile_hook = lambda: _h["hook"]
        sys.modules["antenv.axon_hooks"] = mod
        antenv.axon_hooks = mod
        if "/root/.axon_site" not in sys.path:
            sys.path.insert(0, "/root/.axon_site")
        from trn_agent_boot.trn_boot import _ntff_profile_via_ctypes

        mod.set_axon_ntff_profile_hook(
            _ntff_profile_via_ctypes("/opt/axon/libaxon_pjrt.so")
        )
    import concourse.bass_utils as bu

    bu.upload_artifacts = lambda tmpdir: tmpdir  # no bucket in this sandbox


def kernel(feature, edge_index, W, b):
    global LAST_EXEC_NS, LAST_RESULTS
    from concourse.bass_utils import run_bass_kernel_spmd

    cfg = FULL_CFG
    in_maps, Ts, dcfg = preprocess(feature, edge_index, W, b, cfg)
    nc = build_program(cfg, Ts)
    nc.finalize()
    core_ids = list(range(dcfg["NCORES"]))
    if TRACE:
        try:
            _setup_trace()
            res = run_bass_kernel_spmd(nc, in_maps, core_ids=core_ids, trace=True)
        except Exception as e:
            print(f"[kernel] trace path failed ({type(e).__name__}: {e}); "
                  f"falling back to untraced run", flush=True)
            res = run_bass_kernel_spmd(nc, in_maps, core_ids=core_ids, trace=False)
    else:
        res = run_bass_kernel_spmd(nc, in_maps, core_ids=core_ids, trace=False)
    LAST_EXEC_NS = res.exec_time_ns
    LAST_RESULTS = res
    return postprocess(res.results, dcfg)


# revision 46
# speedup vs baseline: 1.1448x; 1.1448x over previous
"""GCNConv (PyG-style) distributed Bass kernel for 8 TRN2 NeuronCores.

Strategy (edge-parallel by destination, node-partitioned output):
  - Host: deg/dinv via bincount; x' owner-computed on device:
      x'[v] = dinv[v] * (feature[v] @ W)   (per-core node shard, 98 blocks)
  - AllGather x' across the 8 cores (collective_compute).
  - Edges bucketed on host by destination block (128 dest nodes per block).
    Per block: one indirect-DMA gather of the source rows of x', a one-hot
    selection matrix (is_equal vs iota), and matmul-accumulation into PSUM
    implements an exact scatter-add. Self-loop added via identity matmul.
  - Epilogue: *dinv, +b, ReLU -> x_out rows DMA'd out.
  - h = sigmoid(mean(x_out)) computed on host from the full output.
"""

import sys
from contextlib import ExitStack

import numpy as np

if "/opt/trn_rl_repo" not in sys.path:
    sys.path.insert(0, "/opt/trn_rl_repo")

P = 128

FULL_CFG = dict(
    N=100000,
    FIN=256,
    FOUT=128,
    NCORES=8,
)

TRACE = False
LAST_EXEC_NS = None
LAST_RESULTS = None
BF16_TABLE = False  # x_loc/x_gath/gather in bf16 (slower: gpsimd-bound)
BF16_MM = False     # phase-3 matmul operands (oh/gt/ident/xblk) in bf16


def _derive(cfg):
    c = dict(cfg)
    c["NPC"] = -(-c["N"] // c["NCORES"])          # valid nodes per core (ceil)
    c["NBLK"] = -(-c["NPC"] // P)                  # 128-node blocks per core
    c["NPC_PAD"] = c["NBLK"] * P
    c["N_PAD"] = c["NCORES"] * c["NPC_PAD"]
    return c


def preprocess(feature, edge_index, W, b, cfg):
    cfg = _derive(cfg)
    N, NCORES, NPC, NBLK, NPC_PAD = (
        cfg["N"], cfg["NCORES"], cfg["NPC"], cfg["NBLK"], cfg["NPC_PAD"],
    )
    FIN, FOUT = cfg["FIN"], cfg["FOUT"]

    feature = np.ascontiguousarray(feature, dtype=np.float32)
    W = np.ascontiguousarray(W, dtype=np.float32)
    b = np.ascontiguousarray(b, dtype=np.float32)
    row = edge_index[0].astype(np.int64)
    col = edge_index[1].astype(np.int64)

    deg = np.bincount(col, minlength=N).astype(np.float32) + 1.0
    dinv = (1.0 / np.sqrt(deg)).astype(np.float32)

    blk_g = (col // NPC) * NBLK + (col % NPC) // P     # global dest block id
    slot = (col % NPC) % P
    g_of = (row // NPC) * NPC_PAD + (row % NPC)        # gather row (padded layout)

    order = np.argsort(blk_g, kind="stable")
    slot_s = slot[order].astype(np.float32)
    g_s = g_of[order].astype(np.int32)

    cnt = np.bincount(blk_g, minlength=NCORES * NBLK).reshape(NCORES, NBLK)
    Ts = [int(np.ceil(cnt[:, i] / P).max()) for i in range(NBLK)]
    offs = np.concatenate([[0], np.cumsum(Ts)]).astype(int)
    TT = int(offs[-1])

    gidx_h = np.zeros((NCORES, P, max(TT, 1)), np.int32)
    dloc_h = np.full((NCORES, P, max(TT, 1)), -1.0, np.float32)
    bounds = np.concatenate([[0], np.cumsum(cnt.ravel())]).astype(np.int64)
    for c in range(NCORES):
        for i in range(NBLK):
            b0, b1 = bounds[c * NBLK + i], bounds[c * NBLK + i + 1]
            k = b1 - b0
            if k == 0:
                continue
            js = np.arange(k)
            gidx_h[c, js % P, offs[i] + js // P] = g_s[b0:b1]
            dloc_h[c, js % P, offs[i] + js // P] = slot_s[b0:b1]

    bb = np.tile(b[None, :], (P, 1)).astype(np.float32)

    in_maps = []
    for c in range(NCORES):
        lo = c * NPC
        hi = min(lo + NPC, N)
        v = max(hi - lo, 0)
        ft = np.zeros((FIN, NPC_PAD), np.float32)
        if v > 0:
            ft[:, :v] = feature[lo:hi].T
        # pack so each block's [P, 2P] lhsT tile is one contiguous DMA read:
        # ftall[p, i*2P + j*P + c] = featT[j*P + p, i*P + c]
        ftall = np.ascontiguousarray(
            ft.reshape(FIN // P, P, NBLK, P)
            .transpose(1, 2, 0, 3)
            .reshape(P, NBLK * (FIN // P) * P)
        )
        dl = np.zeros(NPC_PAD, np.float32)
        if v > 0:
            dl[:v] = dinv[lo:hi]
        in_maps.append({
            "ftall": ftall,
            "w": W,
            "dinv": np.ascontiguousarray(dl.reshape(NBLK, P).T),
            "bb": bb,
            "gidx": np.ascontiguousarray(gidx_h[c]),
            "dloc": np.ascontiguousarray(dloc_h[c]),
        })
    return in_maps, Ts, cfg


def build_program(cfg, Ts, debug=False):
    import concourse.bacc as bacc
    import concourse.bass as bass
    import concourse.mybir as mybir
    import concourse.tile as tile
    from concourse.masks import make_identity

    cfg = _derive(cfg)
    NCORES, NBLK, NPC_PAD, N_PAD = (
        cfg["NCORES"], cfg["NBLK"], cfg["NPC_PAD"], cfg["N_PAD"],
    )
    FIN, FOUT = cfg["FIN"], cfg["FOUT"]

    Ts = list(Ts)
    offs = np.concatenate([[0], np.cumsum(Ts)]).astype(int)
    TT = max(int(offs[-1]), 1)
    TMAX = max(max(Ts), 1)

    f32 = mybir.dt.float32
    i32 = mybir.dt.int32
    bf16 = mybir.dt.bfloat16
    tdt = bf16 if BF16_TABLE else f32
    mdt = bf16 if BF16_MM else f32

    nc = bacc.Bacc(num_devices=NCORES, num_swdge_queues=4)
    ftall = nc.declare_dram_parameter(
        "ftall", [P, NBLK * (FIN // P) * P], f32, isOutput=False
    )
    w = nc.declare_dram_parameter("w", [FIN, FOUT], f32, isOutput=False)
    dinv = nc.declare_dram_parameter("dinv", [P, NBLK], f32, isOutput=False)
    bb = nc.declare_dram_parameter("bb", [P, FOUT], f32, isOutput=False)
    gidx = nc.declare_dram_parameter("gidx", [P, TT], i32, isOutput=False)
    dloc = nc.declare_dram_parameter("dloc", [P, TT], f32, isOutput=False)
    xout = nc.declare_dram_parameter("xout", [NPC_PAD, FOUT], f32, isOutput=True)
    if debug:
        dbg_iota = nc.declare_dram_parameter(
            "dbg_iota", [P, TMAX, P], f32, isOutput=True)
        dbg_oh = nc.declare_dram_parameter(
            "dbg_oh", [P, TMAX, P], mdt, isOutput=True)
        dbg_gt = nc.declare_dram_parameter(
            "dbg_gt", [P, TMAX, P], mdt, isOutput=True)
        dbg_xall = nc.declare_dram_parameter(
            "dbg_xall", [P, NBLK * FOUT], f32, isOutput=True)
        dbg_xg = nc.declare_dram_parameter(
            "dbg_xg", [4 * P, FOUT], f32, isOutput=True)

    x_loc = nc.dram_tensor("x_loc", [NPC_PAD, FOUT], tdt, kind="Internal")
    x_gath = nc.dram_tensor(
        "x_gath", [N_PAD, FOUT], tdt, kind="Internal", addr_space="Shared"
    )

    with ExitStack() as ctx:
        tc = ctx.enter_context(tile.TileContext(nc))
        const_tp = ctx.enter_context(tc.tile_pool(name="const", bufs=1))
        work_tp = ctx.enter_context(tc.tile_pool(name="work", bufs=2))
        big_tp = ctx.enter_context(tc.tile_pool(name="big", bufs=2))
        psum_tp = ctx.enter_context(tc.tile_pool(name="psum", bufs=2, space="PSUM"))

        # Raw DMA-landed constants.  Every operand a compute engine reads is
        # then re-materialized by a DVE copy: engine instruction structs fit
        # only ONE sync-wait command, so all compute-side deps must coalesce
        # onto a single semaphore (DVE's), with DMA deps carried by the
        # copies (DMACopy descriptors allow multiple waits).
        w_raw = const_tp.tile([P, 2 * FOUT], f32)
        nc.sync.dma_start(out=w_raw[:, 0:FOUT], in_=w[0:P, :])
        nc.sync.dma_start(out=w_raw[:, FOUT:2 * FOUT], in_=w[P:FIN, :])
        dinv_raw = const_tp.tile([P, NBLK], f32)
        nc.sync.dma_start(out=dinv_raw[:], in_=dinv[:, :])
        bb_raw = const_tp.tile([P, FOUT], f32)
        nc.sync.dma_start(out=bb_raw[:], in_=bb[:, :])
        gidx_sb = const_tp.tile([P, TT], i32)
        nc.sync.dma_start(out=gidx_sb[:], in_=gidx[:, :])
        dloc_raw = const_tp.tile([P, TT], f32)
        nc.sync.dma_start(out=dloc_raw[:], in_=dloc[:, :])
        ident_raw = const_tp.tile([P, P], f32)
        make_identity(nc, ident_raw[:])

        w_sb = const_tp.tile([P, 2 * FOUT], f32)
        nc.vector.tensor_copy(out=w_sb[:], in_=w_raw[:])
        dinv_sb = const_tp.tile([P, NBLK], f32)
        nc.vector.tensor_copy(out=dinv_sb[:], in_=dinv_raw[:])
        bb_sb = const_tp.tile([P, FOUT], f32)
        nc.vector.tensor_copy(out=bb_sb[:], in_=bb_raw[:])
        dloc_sb = const_tp.tile([P, TT], mdt)
        nc.vector.tensor_copy(out=dloc_sb[:], in_=dloc_raw[:])
        ident = const_tp.tile([P, P], mdt)
        nc.vector.tensor_copy(out=ident[:], in_=ident_raw[:])
        iota_i = const_tp.tile([P, TMAX, P], i32)
        nc.gpsimd.iota(iota_i[:], pattern=[[0, TMAX], [1, P]], base=0,
                       channel_multiplier=0)
        iota_f = const_tp.tile([P, TMAX, P], f32)
        nc.vector.tensor_copy(out=iota_f[:], in_=iota_i[:])
        if mdt != f32:
            iota_m = const_tp.tile([P, TMAX, P], mdt)
            nc.vector.tensor_copy(out=iota_m[:], in_=iota_f[:])
        else:
            iota_m = iota_f
        xall = const_tp.tile([P, NBLK * FOUT], f32)

        KT = FIN // P

        # phase 1: x'-blocks = dinv * (feat @ W)
        for i in range(NBLK):
            ftr = work_tp.tile([P, KT * P], f32, tag="ftr")
            nc.sync.dma_start(out=ftr[:], in_=ftall[:, i * KT * P:(i + 1) * KT * P])
            ft = work_tp.tile([P, KT * P], f32, tag="ft")
            nc.vector.tensor_copy(out=ft[:], in_=ftr[:])
            px = psum_tp.tile([P, FOUT], f32, tag="px")
            nc.tensor.matmul(out=px[:], lhsT=ft[:, 0:P], rhs=w_sb[:, 0:FOUT],
                             start=True, stop=False)
            nc.tensor.matmul(out=px[:], lhsT=ft[:, P:2 * P], rhs=w_sb[:, FOUT:2 * FOUT],
                             start=False, stop=True)
            xp = xall[:, i * FOUT:(i + 1) * FOUT]
            nc.vector.tensor_scalar_mul(out=xp, in0=px[:], scalar1=dinv_sb[:, i:i + 1])
            if tdt != f32:
                xbf = work_tp.tile([P, FOUT], tdt, tag="xbf")
                nc.vector.tensor_copy(out=xbf[:], in_=xp)
                nc.sync.dma_start(out=x_loc[i * P:(i + 1) * P, :], in_=xbf[:])
            else:
                nc.sync.dma_start(out=x_loc[i * P:(i + 1) * P, :], in_=xp)

        # phase 2: allgather x' across cores
        nc.gpsimd.collective_compute(
            "AllGather",
            mybir.AluOpType.bypass,
            replica_groups=[list(range(NCORES))],
            ins=[x_loc[:, :]],
            outs=[x_gath[:, :]],
        )

        # phase 3: per dest block, gather + one-hot matmul scatter-add.
        # DVE order per block: is_equal -> xblk copy.  The identity matmul
        # reads xblk, so its single DVE wait also covers oh (is_equal), and
        # the t=0 scatter matmul then needs only the gather's DMASW wait.
        for i in range(NBLK):
            T = int(Ts[i])
            off = int(offs[i])
            ps = psum_tp.tile([P, FOUT], f32, tag="agg")
            if T > 0:
                oh = big_tp.tile([P, TMAX, P], mdt, tag="oh")
                nc.vector.tensor_tensor(
                    out=oh[:, 0:T, :],
                    in0=dloc_sb[:, off:off + T].to_broadcast([P, T, P]),
                    in1=iota_m[:, 0:T, :],
                    op=mybir.AluOpType.is_equal,
                )
            xblk = work_tp.tile([P, FOUT], mdt, tag="xblk")
            nc.vector.tensor_copy(out=xblk[:], in_=xall[:, i * FOUT:(i + 1) * FOUT])
            if T > 0:
                gt = big_tp.tile([P, TMAX, P], tdt, tag="gath")
                for t in range(T):
                    nc.gpsimd.indirect_dma_start(
                        out=gt[:, t, :],
                        out_offset=None,
                        in_=x_gath[:, :],
                        in_offset=bass.IndirectOffsetOnAxis(
                            ap=gidx_sb[:, off + t:off + t + 1], axis=0
                        ),
                    )
                if tdt != mdt:
                    gtm = big_tp.tile([P, TMAX, P], mdt, tag="gathm")
                    nc.vector.tensor_copy(out=gtm[:, 0:T, :], in_=gt[:, 0:T, :])
                    gt = gtm
            nc.tensor.matmul(out=ps[:], lhsT=ident[:], rhs=xblk[:],
                             start=True, stop=(T == 0))
            for t in range(T):
                nc.tensor.matmul(
                    out=ps[:],
                    lhsT=oh[:, t:t + 1, :],
                    rhs=gt[:, t:t + 1, :],
                    start=False,
                    stop=(t == T - 1),
                )
            if debug and i == 0:
                nc.sync.dma_start(out=dbg_iota[:, :, :], in_=iota_f[:])
                if T > 0:
                    nc.sync.dma_start(out=dbg_oh[:, 0:T, :], in_=oh[:, 0:T, :])
                    nc.sync.dma_start(out=dbg_gt[:, 0:T, :], in_=gt[:, 0:T, :])
                nc.sync.dma_start(out=dbg_xall[:, :], in_=xall[:])
                nc.sync.dma_start(out=dbg_xg[:, :], in_=x_gath[0:4 * P, :])
            t1 = work_tp.tile([P, FOUT], f32, tag="epi")
            nc.vector.tensor_scalar_mul(out=t1[:], in0=ps[:],
                                        scalar1=dinv_sb[:, i:i + 1])
            nc.vector.tensor_add(out=t1[:], in0=t1[:], in1=bb_sb[:])
            xo = work_tp.tile([P, FOUT], f32, tag="xo")
            nc.vector.tensor_scalar_max(out=xo[:], in0=t1[:], scalar1=0.0)
            nc.sync.dma_start(out=xout[i * P:(i + 1) * P, :], in_=xo[:])

    return nc


def postprocess(results, cfg):
    cfg = _derive(cfg)
    N, NCORES, NPC = cfg["N"], cfg["NCORES"], cfg["NPC"]
    parts = []
    for c in range(NCORES):
        lo = c * NPC
        hi = min(lo + NPC, N)
        if hi > lo:
            parts.append(np.asarray(results[c]["xout"])[: hi - lo])
    x_out = np.concatenate(parts, axis=0).astype(np.float32)
    mean = x_out.mean(axis=0, dtype=np.float64).astype(np.float32)
    h = (1.0 / (1.0 + np.exp(-mean.astype(np.float64)))).astype(np.float32)
    return x_out, h


def _setup_trace():
    """Register the NTFF profile hook that this image's antenv lacks."""
    import types

    try:
        from antenv.axon_hooks import get_axon_ntff_profile_hook  # noqa: F401
    except ImportError:
        import antenv

        mod = types.ModuleType("antenv.axon_hooks")
        _h = {"hook": None}
        mod.set_axon_ntff_profile_hook = lambda h: _h.__setitem__("hook", h)
        mod.get_axon_ntff_profile_hook = lambda: _h["hook"]
        sys.modules["antenv.axon_hooks"] = mod
        antenv.axon_hooks = mod
        if "/root/.axon_site" not in sys.path:
            sys.path.insert(0, "/root/.axon_site")
        from trn_agent_boot.trn_boot import _ntff_profile_via_ctypes

        mod.set_axon_ntff_profile_hook(
            _ntff_profile_via_ctypes("/opt/axon/libaxon_pjrt.so")
        )
    import concourse.bass_utils as bu

    bu.upload_artifacts = lambda tmpdir: tmpdir  # no bucket in this sandbox


def kernel(feature, edge_index, W, b):
    global LAST_EXEC_NS, LAST_RESULTS
    from concourse.bass_utils import run_bass_kernel_spmd

    cfg = FULL_CFG
    in_maps, Ts, dcfg = preprocess(feature, edge_index, W, b, cfg)
    nc = build_program(cfg, Ts)
    nc.finalize()
    core_ids = list(range(dcfg["NCORES"]))
    if TRACE:
        try:
            _setup_trace()
            res = run_bass_kernel_spmd(nc, in_maps, core_ids=core_ids, trace=True)
        except Exception as e:
            print(f"[kernel] trace path failed ({type(e).__name__}: {e}); "
                  f"falling back to untraced run", flush=True)
            res = run_bass_kernel_spmd(nc, in_maps, core_ids=core_ids, trace=False)
    else:
        res = run_bass_kernel_spmd(nc, in_maps, core_ids=core_ids, trace=False)
    LAST_EXEC_NS = res.exec_time_ns
    LAST_RESULTS = res
    return postprocess(res.results, dcfg)
